# revision 1
# baseline (speedup 1.0000x reference)
"""DSSIM loss kernel for Trainium2, 8 NeuronCores.

Strategy:
  - Shard H (2160 rows) across 8 cores: 270 output rows each, input slab of
    280 rows (5-row zero-padded halo on each side, built on host).
  - Host precomputes the 5 blur fields (x, y, x*x, y*y, x*y) in fp32, pads
    W*C with zeros (15 left, right to WPAD), casts to fp16.
  - On device, each 11x11 Gaussian blur is separable:
      pass1 (vertical, over H):  out[wc, h'] = X[h, wc]^T @ Bv[h, h']
        - data tile is the stationary operand => output lands transposed
          (wc on partitions), which is exactly what pass2 needs.
      pass2 (horizontal, over W): out2[wc', h'] = Bh[wc, wc']^T @ T[wc, h']
        - band matrix Bh stationary (one matrix reused everywhere; stride-3
          band because W*C is channel-interleaved).
    xx+yy share one PSUM accumulator through both passes (only bxx+byy is
    needed downstream).
  - Pointwise SSIM math on [98, 270*8] batched tiles, split across
    DVE/ACT/GPSIMD, division via exp(-ln(den)) on ACT tables, final sum via
    tensor_tensor_reduce into per-group accumulator columns.
  - Host sums the per-core accumulators: loss = 1 - sum/(H*W*C).
"""

import sys

sys.path.insert(0, "/opt/trn_rl_repo")

import numpy as np

import concourse.bass as bass
import concourse.bacc as bacc
import concourse.tile as tile
from concourse import mybir
from concourse.bass_utils import run_bass_kernel_spmd

# ---------------- problem geometry (hardcoded) ----------------
H, W, C = 2160, 3840, 3
WC = W * C  # 11520
NCORES = 8
HOUT = H // NCORES  # 270
HSLAB = HOUT + 10  # 280 input rows per core (5 halo each side)
PADL = 15  # left zero pad in wc (= 5 pixels * 3 channels)
TW = 98  # output tile width in wc' (128 - 2*15)
NT = (WC + TW - 1) // TW  # 118 tiles; last tile has 54 valid columns
WPAD = TW * (NT - 1) + 128  # 11594 -> pad a bit for alignment
WPAD = ((WPAD + 7) // 8) * 8  # 11600
TGROUP = 8
NB = (NT + TGROUP - 1) // TGROUP  # 15 groups
NV = HOUT  # 270 (h' extent, free dim of pass1/pass2 outputs)
WIN, SIGMA = 11, 1.5
C1, C2 = 0.01**2, 0.03**2
# vertical conv blocks: (slab_row_start, K, hprime_start, M)
BLOCKS = [(0, 128, 0, 118), (118, 128, 118, 118), (236, 44, 236, 34)]

F16 = mybir.dt.float16
F32 = mybir.dt.float32

FIELDS = ("x", "y", "s", "xy")  # s = xx + yy merged in PSUM


def _gauss():
    co = np.arange(WIN, dtype=np.float64) - (WIN // 2)
    g = np.exp(-(co**2) / (2.0 * SIGMA**2))
    return (g / g.sum()).astype(np.float32)


def _band_gain():
    # f16-rounded taps don't sum to exactly 1; each blur pass applies gain s.
    g16 = _gauss().astype(np.float16).astype(np.float64)
    return float(g16.sum())


def _bands():
    g = _gauss()
    bv = np.zeros((128, 118), dtype=np.float32)
    for k in range(128):
        for m in range(118):
            t = k - m
            if 0 <= t <= 10:
                bv[k, m] = g[t]
    bh = np.zeros((128, TW), dtype=np.float32)
    for k in range(128):
        for m in range(TW):
            d3 = k - m - 15
            if d3 % 3 == 0 and -15 <= d3 <= 15:
                bh[k, m] = g[d3 // 3 + 5]
    return bv.astype(np.float16), bh.astype(np.float16)


def _body(tc, ins, outs):
    nc = tc.nc
    fx, fy, fxx, fyy, fxy = (ins[k] for k in ("fx", "fy", "fxx", "fyy", "fxy"))
    bv_d, bh_d = ins["bv"], ins["bh"]
    acc_out = outs["acc"]
    ADD, SUB, MUL = (
        mybir.AluOpType.add,
        mybir.AluOpType.subtract,
        mybir.AluOpType.mult,
    )
    ACTF = mybir.ActivationFunctionType
    s = _band_gain()
    SCL = 1.0 / (s * s)  # cancel the per-pass f16 band gain (applied twice)

    consts = tc.alloc_tile_pool(name="consts", bufs=1)
    inp = tc.alloc_tile_pool(name="inp", bufs=2)
    tvp = tc.alloc_tile_pool(name="tv", bufs=2)
    pvp = tc.alloc_tile_pool(name="pv", bufs=1, space="PSUM")
    p2p = tc.alloc_tile_pool(name="p2", bufs=1, space="PSUM")
    sbp = tc.alloc_tile_pool(name="sb", bufs=2)
    pwp = tc.alloc_tile_pool(name="pw", bufs=2)

    bv_s = consts.tile([128, 118], F16)
    nc.sync.dma_start(out=bv_s, in_=bv_d[:, :])
    bh_s = consts.tile([128, TW], F16)
    nc.sync.dma_start(out=bh_s, in_=bh_d[:, :])
    acc = consts.tile([128, NB], F32)
    nc.vector.memset(acc, 0.0)
    msk_s = consts.tile([128, 2], F32)
    nc.sync.dma_start(out=msk_s, in_=ins["msk"][:, :])

    WGMAX = TW * TGROUP + 30  # 814

    for grp in range(NB):
        t0 = grp * TGROUP
        nt = min(TGROUP, NT - t0)
        wg0 = TW * t0
        WG = TW * nt + 30
        W2 = NV * nt

        # ---- load input strips: 5 fields x 3 overlapping row-blocks ----
        itiles = {}
        for fname, fap in (("x", fx), ("y", fy), ("xx", fxx), ("yy", fyy), ("xy", fxy)):
            for bi, (rs, K, _, _) in enumerate(BLOCKS):
                t = inp.tile([128, WGMAX], F16, tag=f"in_{fname}_{bi}", name=f"in_{fname}_{bi}_{grp}")
                nc.sync.dma_start(out=t[:K, :WG], in_=fap[rs : rs + K, wg0 : wg0 + WG])
                itiles[(fname, bi)] = t

        # sb batch buffers (pass2 results, fp16)
        sb = {f: sbp.tile([TW, NV * TGROUP], F16, tag=f"sb_{f}", name=f"sb_{f}_{grp}") for f in FIELDS}

        for ti in range(nt):
            o = TW * ti
            o2 = NV * ti
            # ---- pass1: vertical blur, transposed output [wc=128, h'=270]
            pv = {f: pvp.tile([128, NV], F32, tag=f"pv_{f}", name=f"pv_{f}_{grp}_{ti}") for f in FIELDS}
            for f, srcs in (("x", ("x",)), ("y", ("y",)), ("s", ("xx", "yy")), ("xy", ("xy",))):
                nsrc = len(srcs)
                for si, sname in enumerate(srcs):
                    for bi, (rs, K, hp, M) in enumerate(BLOCKS):
                        nc.tensor.matmul(
                            pv[f][:, hp : hp + M],
                            itiles[(sname, bi)][:K, o : o + 128],
                            bv_s[:K, :M],
                            start=(bi == 0 and si == 0),
                            stop=(bi == 2 and si == nsrc - 1),
                        )
            # ---- evacuate PSUM -> SBUF fp16 (split DVE / ACT)
            tv = {f: tvp.tile([128, NV], F16, tag=f"tv_{f}", name=f"tv_{f}_{grp}_{ti}") for f in FIELDS}
            nc.vector.tensor_scalar(tv["x"], pv["x"], SCL, None, MUL)
            nc.vector.tensor_scalar(tv["xy"], pv["xy"], SCL, None, MUL)
            nc.scalar.activation(tv["y"], pv["y"], ACTF.Copy, scale=SCL)
            nc.scalar.activation(tv["s"], pv["s"], ACTF.Copy, scale=SCL)
            # ---- pass2: horizontal blur [wc'=98, h'=270]
            p2 = {f: p2p.tile([TW, NV], F32, tag=f"p2_{f}", name=f"p2_{f}_{grp}_{ti}") for f in FIELDS}
            for f in FIELDS:
                nc.tensor.matmul(p2[f], bh_s, tv[f], start=True, stop=True)
            # ---- evacuate to batch buffers fp16 (split DVE / ACT)
            nc.vector.tensor_scalar(sb["x"][:, o2 : o2 + NV], p2["x"], 0.0, None, ADD)
            nc.vector.tensor_scalar(sb["xy"][:, o2 : o2 + NV], p2["xy"], 0.0, None, ADD)
            nc.scalar.copy(out=sb["y"][:, o2 : o2 + NV], in_=p2["y"])
            nc.scalar.copy(out=sb["s"][:, o2 : o2 + NV], in_=p2["s"])

        # ---- pointwise SSIM on [98, W2] ----
        bx, by, S, bxy = sb["x"], sb["y"], sb["s"], sb["xy"]
        mx2 = pwp.tile([TW, NV * TGROUP], F16, tag="mx2", name=f"mx2_{grp}")
        my2 = pwp.tile([TW, NV * TGROUP], F16, tag="my2", name=f"my2_{grp}")
        mxy = pwp.tile([TW, NV * TGROUP], F16, tag="mxy", name=f"mxy_{grp}")
        n1t = pwp.tile([TW, NV * TGROUP], F16, tag="n1t", name=f"n1t_{grp}")
        lnt = pwp.tile([TW, NV * TGROUP], F32, tag="lnt", name=f"lnt_{grp}")

        # squares on ACT, products/sums on GPSIMD+DVE
        nc.scalar.activation(mx2[:, :W2], bx[:, :W2], ACTF.Square)
        nc.scalar.activation(my2[:, :W2], by[:, :W2], ACTF.Square)
        nc.gpsimd.tensor_tensor(mxy[:, :W2], bx[:, :W2], by[:, :W2], MUL)
        # u = mx2 + my2 (in-place into mx2)
        nc.gpsimd.tensor_tensor(mx2[:, :W2], mx2[:, :W2], my2[:, :W2], ADD)
        # sxyp = bxy - mxy (in-place into bxy)
        nc.gpsimd.tensor_tensor(bxy[:, :W2], bxy[:, :W2], mxy[:, :W2], SUB)
        # N1' = mxy + C1/2
        nc.vector.tensor_scalar(n1t[:, :W2], mxy[:, :W2], C1 / 2, None, ADD)
        # num = (sxyp + C2/2) * N1'   (in mxy)
        nc.vector.scalar_tensor_tensor(
            out=mxy[:, :W2], in0=bxy[:, :W2], scalar=C2 / 2, in1=n1t[:, :W2],
            op0=ADD, op1=MUL,
        )
        # D2m = (u - C2) - S = -(sxx+syy+C2)   (in S)
        nc.vector.scalar_tensor_tensor(
            out=S[:, :W2], in0=mx2[:, :W2], scalar=C2, in1=S[:, :W2],
            op0=SUB, op1=SUB,
        )
        # denm = (u + C1) * D2m = -den   (in mx2)
        nc.vector.scalar_tensor_tensor(
            out=mx2[:, :W2], in0=mx2[:, :W2], scalar=C1, in1=S[:, :W2],
            op0=ADD, op1=MUL,
        )
        if t0 + nt == NT:
            # last output tile (t=117) has only 54 valid wc' columns; mask out
            # the invalid region (partitions 54.., local ti = NT-1-t0):
            #   num <- num * mask            (mask: 1 valid / 0 invalid)
            #   denm <- denm * mask + (mask - 1)   (-> -1 on invalid, so
            #            ln(-denm)=0, rec=1, contribution = num*rec = 0)
            iv0 = NV * (NT - 1 - t0)
            nc.vector.tensor_scalar(
                mxy[:TW, iv0 : iv0 + NV], mxy[:TW, iv0 : iv0 + NV],
                msk_s[:TW, 0:1], None, MUL,
            )
            nc.vector.tensor_scalar(
                mx2[:TW, iv0 : iv0 + NV], mx2[:TW, iv0 : iv0 + NV],
                msk_s[:TW, 0:1], msk_s[:TW, 1:2], MUL, ADD,
            )
        # ln(den) = Ln(-denm), then 1/den = Exp(-ln)
        nc.scalar.activation(lnt[:, :W2], mx2[:, :W2], ACTF.Ln, scale=-1.0)
        nc.scalar.activation(my2[:, :W2], lnt[:, :W2], ACTF.Exp, scale=-1.0)
        # ssim = (num*4) * rec ; acc[:, grp] = sum(ssim) per partition
        nc.vector.scalar_tensor_tensor(
            out=bx[:, :W2], in0=mxy[:, :W2], scalar=4.0, in1=my2[:, :W2],
            op0=MUL, op1=MUL, accum_out=acc[:TW, grp : grp + 1],
        )

    nc.sync.dma_start(out=acc_out[:, :], in_=acc)

    for p in (pwp, sbp, p2p, pvp, tvp, inp, consts):
        p.release()


_CACHE = {}


def _get_compiled():
    if "nc" in _CACHE:
        return _CACHE["nc"], _CACHE["aps"]
    nc = bacc.Bacc("TRN2", target_bir_lowering=False, debug=False, num_devices=NCORES)
    ins = {}
    for name in ("fx", "fy", "fxx", "fyy", "fxy"):
        ins[name] = nc.dram_tensor(name, [HSLAB, WPAD], F16, kind="ExternalInput").ap()
    ins["bv"] = nc.dram_tensor("bv", [128, 118], F16, kind="ExternalInput").ap()
    ins["bh"] = nc.dram_tensor("bh", [128, TW], F16, kind="ExternalInput").ap()
    ins["msk"] = nc.dram_tensor("msk", [128, 2], F32, kind="ExternalInput").ap()
    outs = {"acc": nc.dram_tensor("acc", [128, NB], F32, kind="ExternalOutput").ap()}
    with tile.TileContext(nc) as tc:
        _body(tc, ins, outs)
    nc.compile()
    _CACHE["nc"] = nc
    _CACHE["aps"] = (ins, outs)
    return nc, (ins, outs)


LAST_RES = None


def kernel(X, Y, _trace=False, _trace_kwargs=None):
    global LAST_RES
    X = np.asarray(X, dtype=np.float32).reshape(H, WC)
    Y = np.asarray(Y, dtype=np.float32).reshape(H, WC)

    bv, bh = _bands()
    nvalid = WC - TW * (NT - 1)  # 54
    msk = np.zeros((128, 2), dtype=np.float32)
    msk[:nvalid, 0] = 1.0
    msk[:, 1] = msk[:, 0] - 1.0
    fields = {
        "fx": X,
        "fy": Y,
        "fxx": X * X,
        "fyy": Y * Y,
        "fxy": X * Y,
    }
    # pad rows (5 top/bottom) and wc (15 left, to WPAD right), cast fp16
    padded = {}
    for k, a in fields.items():
        p = np.zeros((H + 10, WPAD), dtype=np.float16)
        p[5 : 5 + H, PADL : PADL + WC] = a.astype(np.float16)
        padded[k] = p

    in_maps = []
    for c in range(NCORES):
        m = {k: np.ascontiguousarray(p[HOUT * c : HOUT * c + HSLAB]) for k, p in padded.items()}
        m["bv"] = bv
        m["bh"] = bh
        m["msk"] = msk
        in_maps.append(m)

    nc, _ = _get_compiled()
    res = run_bass_kernel_spmd(
        nc, in_maps, core_ids=list(range(NCORES)),
        trace=_trace, **(_trace_kwargs or {}),
    )
    LAST_RES = res
    total = 0.0
    for r in res.results:
        total += float(np.asarray(r["acc"])[:TW, :].astype(np.float64).sum())
    loss = 1.0 - total / (H * W * C)
    return np.float32(loss)



# revision 2
# speedup vs baseline: 1.2739x; 1.2739x over previous
"""DSSIM loss kernel for Trainium2, 8 NeuronCores — v2.

Strategy vs v1 baseline (705us), per trace analysis:
  - Host fields {x+y, x-y, 2xy, x^2+y^2}: the sigma algebra becomes linear
    in the blurred fields, so the pointwise stage is 7 TT + 2 TS + 1 accum
    on DVE/GPSIMD (v1 used scalar_tensor_tensor at ~4us each).
  - ACT uses only Copy/Square/Reciprocal — all live in the single
    `reciprocal_and_small` activation-table set: removes 31 ACT_TABLE_LOADs
    (47us + serialization) and does the division in one pass (v1: Ln+Exp).
    Reciprocal is emitted raw (bass guards it for accuracy; DSSIM slack is
    huge: ssim~0.007, so 1% recip error moves the loss ~7e-5).
  - All pointwise math scaled by 8 (folded into free scale slots) so 1/den
    stays < ~250 and fits fp16 comfortably.
  - PSUM: both passes tiled in h'-halves (135) with two fields packed per
    2KB bank; pv and p2 pools double-buffered (4+4 banks) so the tensor
    engine never waits on evacuations (v1: 529us MATMUL semaphore wait,
    single-buffered PSUM).
  - pass1 evac: ACT Copy, one instr per packed field-pair.
  - pass2 "evac" fused with math: Square(psum) on ACT for the mean fields,
    one 2-op tensor_scalar from PSUM on DVE for the second-moment fields.
  - Final sum via tensor_scalar accum_out into per-group acc columns.
"""

import sys

sys.path.insert(0, "/opt/trn_rl_repo")

import numpy as np

import concourse.bass as bass
import concourse.bacc as bacc
import concourse.tile as tile
from concourse import mybir
from concourse.bass_utils import run_bass_kernel_spmd

# ---------------- problem geometry (hardcoded) ----------------
H, W, C = 2160, 3840, 3
WC = W * C  # 11520
NCORES = 8
HOUT = H // NCORES  # 270
HSLAB = HOUT + 10  # 280 input rows per core (5 halo each side)
PADL = 15  # left zero pad in wc (= 5 pixels * 3 channels)
TW = 98  # output tile width in wc' (128 - 2*15)
NT = (WC + TW - 1) // TW  # 118 tiles; last tile has 54 valid columns
WPAD = TW * (NT - 1) + 128  # 11594
WPAD = ((WPAD + 7) // 8) * 8  # 11600
TGROUP = 8
NB = (NT + TGROUP - 1) // TGROUP  # 15 groups
NV = HOUT  # 270 (h' extent)
HNV = NV // 2  # 135 (h' half)
WIN, SIGMA = 11, 1.5
C1, C2 = 0.01**2, 0.03**2
SC = 8.0  # range scale folded into num and den (cancels in the ratio)

# input row-blocks, aligned with the h'-half matmul structure:
#   half 0 (h' 0..134)   <- rows 0..144:   tile A rows [0,128) + tile B rows [118,145)
#   half 1 (h' 135..269) <- rows 135..279: tile C rows [135,263) + tile D rows [253,280)
IBLOCKS = [(0, 128), (118, 27), (135, 128), (253, 27)]
# per half: list of (input_block_idx, K, out_col, M)
HBLOCKS = {
    0: [(0, 128, 0, 118), (1, 27, 118, 17)],
    1: [(2, 128, 0, 118), (3, 27, 118, 17)],
}

F16 = mybir.dt.float16
F32 = mybir.dt.float32

FIELDS = ("p", "m", "q", "s")  # x+y, x-y, 2xy, x^2+y^2


def _gauss():
    co = np.arange(WIN, dtype=np.float64) - (WIN // 2)
    g = np.exp(-(co**2) / (2.0 * SIGMA**2))
    return (g / g.sum()).astype(np.float32)


def _band_gain():
    g16 = _gauss().astype(np.float16).astype(np.float64)
    return float(g16.sum())


def _bands():
    g = _gauss()
    bv = np.zeros((128, 118), dtype=np.float32)
    for k in range(128):
        for m in range(118):
            t = k - m
            if 0 <= t <= 10:
                bv[k, m] = g[t]
    bh = np.zeros((128, TW), dtype=np.float32)
    for k in range(128):
        for m in range(TW):
            d3 = k - m - 15
            if d3 % 3 == 0 and -15 <= d3 <= 15:
                bh[k, m] = g[d3 // 3 + 5]
    return bv.astype(np.float16), bh.astype(np.float16)


def _act_raw(eng, out, in_, func, scale=1.0):
    """Emit InstActivation directly (bypasses the bass Reciprocal guard)."""
    ins = [
        eng.lower_ap(in_),
        mybir.ImmediateValue(dtype=mybir.dt.float32, value=0.0),  # bias
        mybir.ImmediateValue(dtype=mybir.dt.float32, value=scale),  # scale
        mybir.ImmediateValue(dtype=mybir.dt.float32, value=0.0),  # alpha
    ]
    return eng.add_instruction(
        mybir.InstActivation(
            name=eng.bass.get_next_instruction_name(),
            func=func,
            ins=ins,
            outs=[eng.lower_ap(out)],
        )
    )


def _pair_view(ap2d, width):
    """[P, 2*width] -> [P, 2, width] (field-pair view)."""
    return ap2d.rearrange("p (f v) -> p f v", v=width)


def _body(tc, ins, outs):
    nc = tc.nc
    fin = {f: ins[f"f{f}"] for f in FIELDS}
    bv_d, bh_d = ins["bv"], ins["bh"]
    acc_out = outs["acc"]
    ADD, SUB, MUL = (
        mybir.AluOpType.add,
        mybir.AluOpType.subtract,
        mybir.AluOpType.mult,
    )
    ACTF = mybir.ActivationFunctionType
    s = _band_gain()
    SCL = 1.0 / (s * s)  # cancel per-pass f16 band gain at pass1 evac
    KSQ = float(np.sqrt(SC / 2.0))  # Square scale: (KSQ*m)^2 = SC*m^2/2
    C1S = SC * C1
    C2S = SC * C2

    consts = tc.alloc_tile_pool(name="consts", bufs=1)
    inp = tc.alloc_tile_pool(name="inp", bufs=2)
    tvp = tc.alloc_tile_pool(name="tv", bufs=3)
    pvp = tc.alloc_tile_pool(name="pv", bufs=2, space="PSUM")
    p2p = tc.alloc_tile_pool(name="p2", bufs=2, space="PSUM")
    sbp = tc.alloc_tile_pool(name="sb", bufs=2)
    pwp = tc.alloc_tile_pool(name="pw", bufs=2)

    bv_s = consts.tile([128, 118], F16)
    nc.sync.dma_start(out=bv_s, in_=bv_d[:, :])
    bh_s = consts.tile([128, TW], F16)
    nc.sync.dma_start(out=bh_s, in_=bh_d[:, :])
    acc = consts.tile([128, NB], F32)
    nc.vector.memset(acc, 0.0)
    msk_s = consts.tile([128, 2], F32)
    nc.sync.dma_start(out=msk_s, in_=ins["msk"][:, :])

    WGMAX = TW * TGROUP + 30  # 814
    GW = NV * TGROUP  # 2160 columns per field in group buffers

    for grp in range(NB):
        t0 = grp * TGROUP
        nt = min(TGROUP, NT - t0)
        wg0 = TW * t0
        WG = TW * nt + 30
        W2 = NV * nt

        # ---- load input strips: 4 fields x 4 row-blocks ----
        itiles = {}
        for fname in FIELDS:
            for bi, (rs, K) in enumerate(IBLOCKS):
                t = inp.tile(
                    [128, WGMAX], F16, tag=f"in_{fname}_{bi}",
                    name=f"in_{fname}_{bi}_{grp}",
                )
                nc.sync.dma_start(
                    out=t[:K, :WG], in_=fin[fname][rs : rs + K, wg0 : wg0 + WG]
                )
                itiles[(fname, bi)] = t

        # group buffers: field A in cols [0,GW), field B in cols [GW,2GW)
        sbAB = sbp.tile([TW, 2 * GW], F16, tag="sbAB", name=f"sbAB_{grp}")  # A'|B'
        sb34 = sbp.tile([TW, 2 * GW], F16, tag="sb34", name=f"sb34_{grp}")  # S3'|S4'

        for ti in range(nt):
            o = TW * ti
            # tv: pass1 results, [128, 2*NV] per field pair
            tvAB = tvp.tile([128, 2 * NV], F16, tag="tvAB", name=f"tvAB_{grp}_{ti}")
            tvCD = tvp.tile([128, 2 * NV], F16, tag="tvCD", name=f"tvCD_{grp}_{ti}")
            for h in (0, 1):
                hs = h * HNV
                # ---- pass1 (vertical blur), h'-half, fields packed in pairs
                pvm = pvp.tile([128, NV], F32, tag="pvm", name=f"pvm_{grp}_{ti}_{h}")
                pvs = pvp.tile([128, NV], F32, tag="pvs", name=f"pvs_{grp}_{ti}_{h}")
                for pv_t, fpair in ((pvm, ("p", "m")), (pvs, ("q", "s"))):
                    for fi, fname in enumerate(fpair):
                        fc = fi * HNV
                        for bi, K, hp, M in HBLOCKS[h]:
                            nc.tensor.matmul(
                                pv_t[:, fc + hp : fc + hp + M],
                                itiles[(fname, bi)][:K, o : o + 128],
                                bv_s[:K, :M],
                                start=True,
                                stop=True,
                            )
                # ---- pass1 evac on ACT: one Copy per packed pair ----
                # tv[:, f*NV + hs : +HNV] <- pv[:, f*HNV : +HNV] for f in 0,1
                nc.scalar.activation(
                    _pair_view(tvAB, NV)[:, :, hs : hs + HNV],
                    _pair_view(pvm, HNV),
                    ACTF.Copy,
                    scale=SCL,
                )
                nc.scalar.activation(
                    _pair_view(tvCD, NV)[:, :, hs : hs + HNV],
                    _pair_view(pvs, HNV),
                    ACTF.Copy,
                    scale=SCL,
                )
            for h in (0, 1):
                hs = h * HNV
                # ---- pass2 (horizontal blur), h'-half, packed outputs ----
                p2m = p2p.tile([TW, NV], F32, tag="p2m", name=f"p2m_{grp}_{ti}_{h}")
                p2s = p2p.tile([TW, NV], F32, tag="p2s", name=f"p2s_{grp}_{ti}_{h}")
                for p2_t, tv_t in ((p2m, tvAB), (p2s, tvCD)):
                    for fi in (0, 1):
                        nc.tensor.matmul(
                            p2_t[:, fi * HNV : (fi + 1) * HNV],
                            bh_s,
                            tv_t[:, fi * NV + hs : fi * NV + hs + HNV],
                            start=True,
                            stop=True,
                        )
                # ---- pass2 evac fused with first pointwise layer ----
                colA = ti * NV + hs
                # A'|B' = (KSQ * mean)^2 = SC/2 * mean^2   [ACT Square]
                nc.scalar.activation(
                    _pair_view(sbAB, GW)[:, :, colA : colA + HNV],
                    _pair_view(p2m, HNV),
                    ACTF.Square,
                    scale=KSQ,
                )
                # S3'|S4' = SC * blur + SC*C2   [DVE 2-op tensor_scalar]
                nc.vector.tensor_scalar(
                    _pair_view(sb34, GW)[:, :, colA : colA + HNV],
                    _pair_view(p2s, HNV),
                    SC,
                    C2S,
                    MUL,
                    ADD,
                )

        # ---- group pointwise stage on [98, W2] fp16 ----
        Av = sbAB[:, 0:W2]
        Bv = sbAB[:, GW : GW + W2]
        S3 = sb34[:, 0:W2]
        S4 = sb34[:, GW : GW + W2]
        al0 = pwp.tile([TW, GW], F16, tag="al0", name=f"al0_{grp}")
        u0t = pwp.tile([TW, GW], F16, tag="u0t", name=f"u0t_{grp}")
        alt = pwp.tile([TW, GW], F16, tag="alt", name=f"alt_{grp}")
        u1t = pwp.tile([TW, GW], F16, tag="u1t", name=f"u1t_{grp}")
        rnt = pwp.tile([TW, GW], F16, tag="rnt", name=f"rnt_{grp}")
        rdt = pwp.tile([TW, GW], F16, tag="rdt", name=f"rdt_{grp}")
        numt = pwp.tile([TW, GW], F16, tag="numt", name=f"numt_{grp}")
        dent = pwp.tile([TW, GW], F16, tag="dent", name=f"dent_{grp}")
        rect = pwp.tile([TW, GW], F16, tag="rect", name=f"rect_{grp}")

        # al0 = A' - B' = SC * mux*muy              (DVE)
        nc.vector.tensor_tensor(al0[:, :W2], Av, Bv, SUB)
        # u0 = A' + B' = SC/2 * (mux^2 + muy^2)     (GPSIMD)
        nc.gpsimd.tensor_tensor(u0t[:, :W2], Av, Bv, ADD)
        # rn = S3' - 2*al0 ... careful with factors, see note below
        # S3' = SC*(2*blur(xy)) + SC*C2 ; al0 = SC*(2 mux muy)/1 ?
        # al0 = (SC/2)(m1^2 - m2^2)/1 = SC/2 * 4 mux muy = 2*SC*mux*muy
        # => rn = S3' - al0 = SC*(2 sigxy + C2)     (DVE)
        nc.vector.tensor_tensor(rnt[:, :W2], S3, al0[:, :W2], SUB)
        # al = al0 + SC*C1 = SC*(2 mux muy + C1)    (DVE TS)
        nc.vector.tensor_scalar(alt[:, :W2], al0[:, :W2], 1.0, C1S, MUL, ADD)
        # rd = S4' - u0 = SC*(sigxx + sigyy + C2)   (GPSIMD, after u0)
        nc.gpsimd.tensor_tensor(rdt[:, :W2], S4, u0t[:, :W2], SUB)
        # u1 = u0 + SC*C1 = SC*(mux^2+muy^2 + C1)   (DVE TS)
        nc.vector.tensor_scalar(u1t[:, :W2], u0t[:, :W2], 1.0, C1S, MUL, ADD)
        # num = al * rn                              (DVE)
        nc.vector.tensor_tensor(numt[:, :W2], alt[:, :W2], rnt[:, :W2], MUL)
        # den = u1 * rd                              (DVE)
        nc.vector.tensor_tensor(dent[:, :W2], u1t[:, :W2], rdt[:, :W2], MUL)
        if t0 + nt == NT:
            # last output tile (t=117): only 54 valid wc' partitions.
            # num <- num*m ; den <- den*m + (1-m)  (m: 1 valid / 0 invalid)
            iv0 = NV * (NT - 1 - t0)
            nc.vector.tensor_scalar(
                numt[:TW, iv0 : iv0 + NV], numt[:TW, iv0 : iv0 + NV],
                msk_s[:TW, 0:1], 0.0, MUL, ADD,
            )
            nc.vector.tensor_scalar(
                dent[:TW, iv0 : iv0 + NV], dent[:TW, iv0 : iv0 + NV],
                msk_s[:TW, 0:1], msk_s[:TW, 1:2], MUL, ADD,
            )
        # rec = 1/den                                (ACT Reciprocal, raw)
        _act_raw(nc.scalar, rect[:, :W2], dent[:, :W2], ACTF.Reciprocal)
        # ssim = num * rec; acc[:, grp] += sum      (DVE TT + TS accum)
        nc.vector.tensor_tensor(numt[:, :W2], numt[:, :W2], rect[:, :W2], MUL)
        nc.vector.tensor_scalar(
            al0[:, :W2], numt[:, :W2], 1.0, 0.0, MUL, ADD,
            accum_out=acc[:TW, grp : grp + 1],
        )

    nc.sync.dma_start(out=acc_out[:, :], in_=acc)

    for p in (pwp, sbp, p2p, pvp, tvp, inp, consts):
        p.release()


_CACHE = {}


def _get_compiled():
    if "nc" in _CACHE:
        return _CACHE["nc"], _CACHE["aps"]
    nc = bacc.Bacc("TRN2", target_bir_lowering=False, debug=False, num_devices=NCORES)
    ins = {}
    for f in FIELDS:
        ins[f"f{f}"] = nc.dram_tensor(
            f"f{f}", [HSLAB, WPAD], F16, kind="ExternalInput"
        ).ap()
    ins["bv"] = nc.dram_tensor("bv", [128, 118], F16, kind="ExternalInput").ap()
    ins["bh"] = nc.dram_tensor("bh", [128, TW], F16, kind="ExternalInput").ap()
    ins["msk"] = nc.dram_tensor("msk", [128, 2], F32, kind="ExternalInput").ap()
    outs = {"acc": nc.dram_tensor("acc", [128, NB], F32, kind="ExternalOutput").ap()}
    with tile.TileContext(nc) as tc:
        _body(tc, ins, outs)
    nc.compile()
    _CACHE["nc"] = nc
    _CACHE["aps"] = (ins, outs)
    return nc, (ins, outs)


LAST_RES = None


def kernel(X, Y, _trace=False, _trace_kwargs=None):
    global LAST_RES
    X = np.asarray(X, dtype=np.float32).reshape(H, WC)
    Y = np.asarray(Y, dtype=np.float32).reshape(H, WC)

    bv, bh = _bands()
    nvalid = WC - TW * (NT - 1)  # 54
    msk = np.zeros((128, 2), dtype=np.float32)
    msk[:nvalid, 0] = 1.0
    msk[:, 1] = 1.0 - msk[:, 0]
    fields = {
        "fp": X + Y,
        "fm": X - Y,
        "fq": 2.0 * (X * Y),
        "fs": X * X + Y * Y,
    }
    # pad rows (5 top/bottom) and wc (15 left, to WPAD right), cast fp16
    padded = {}
    for k, a in fields.items():
        p = np.zeros((H + 10, WPAD), dtype=np.float16)
        p[5 : 5 + H, PADL : PADL + WC] = a.astype(np.float16)
        padded[k] = p

    in_maps = []
    for c in range(NCORES):
        m = {
            k: np.ascontiguousarray(p[HOUT * c : HOUT * c + HSLAB])
            for k, p in padded.items()
        }
        m["bv"] = bv
        m["bh"] = bh
        m["msk"] = msk
        in_maps.append(m)

    nc, _ = _get_compiled()
    res = run_bass_kernel_spmd(
        nc, in_maps, core_ids=list(range(NCORES)),
        trace=_trace, **(_trace_kwargs or {}),
    )
    LAST_RES = res
    total = 0.0
    for r in res.results:
        total += float(np.asarray(r["acc"])[:TW, :].astype(np.float64).sum())
    # acc holds SC*num/(SC*den) = ssim, summed over all pixels
    loss = 1.0 - total / (H * W * C)
    return np.float32(loss)


# revision 3
# speedup vs baseline: 1.2956x; 1.0170x over previous
"""DSSIM loss kernel for Trainium2, 8 NeuronCores — v2.

Strategy vs v1 baseline (705us), per trace analysis:
  - Host fields {x+y, x-y, 2xy, x^2+y^2}: the sigma algebra becomes linear
    in the blurred fields, so the pointwise stage is 7 TT + 2 TS + 1 accum
    on DVE/GPSIMD (v1 used scalar_tensor_tensor at ~4us each).
  - ACT uses only Copy/Square/Reciprocal — all live in the single
    `reciprocal_and_small` activation-table set: removes 31 ACT_TABLE_LOADs
    (47us + serialization) and does the division in one pass (v1: Ln+Exp).
    Reciprocal is emitted raw (bass guards it for accuracy; DSSIM slack is
    huge: ssim~0.007, so 1% recip error moves the loss ~7e-5).
  - All pointwise math scaled by 8 (folded into free scale slots) so 1/den
    stays < ~250 and fits fp16 comfortably.
  - PSUM: both passes tiled in h'-halves (135) with two fields packed per
    2KB bank; pv and p2 pools double-buffered (4+4 banks) so the tensor
    engine never waits on evacuations (v1: 529us MATMUL semaphore wait,
    single-buffered PSUM).
  - pass1 evac: ACT Copy, one instr per packed field-pair.
  - pass2 "evac" fused with math: Square(psum) on ACT for the mean fields,
    one 2-op tensor_scalar from PSUM on DVE for the second-moment fields.
  - Final sum via tensor_scalar accum_out into per-group acc columns.
"""

import sys

sys.path.insert(0, "/opt/trn_rl_repo")

import numpy as np

import concourse.bass as bass
import concourse.bacc as bacc
import concourse.tile as tile
from concourse import mybir
from concourse.bass_utils import run_bass_kernel_spmd

# ---------------- problem geometry (hardcoded) ----------------
H, W, C = 2160, 3840, 3
WC = W * C  # 11520
NCORES = 8
HOUT = H // NCORES  # 270
HSLAB = HOUT + 10  # 280 input rows per core (5 halo each side)
PADL = 15  # left zero pad in wc (= 5 pixels * 3 channels)
TW = 98  # output tile width in wc' (128 - 2*15)
NT = (WC + TW - 1) // TW  # 118 tiles; last tile has 54 valid columns
WPAD = TW * (NT - 1) + 128  # 11594
WPAD = ((WPAD + 7) // 8) * 8  # 11600
TGROUP = 8
NB = (NT + TGROUP - 1) // TGROUP  # 15 groups
NV = HOUT  # 270 (h' extent)
HNV = NV // 2  # 135 (h' half)
WIN, SIGMA = 11, 1.5
C1, C2 = 0.01**2, 0.03**2
SC = 8.0  # range scale folded into num and den (cancels in the ratio)

# input row-blocks (baseline 3-block layout):
IBLOCKS = [(0, 128), (118, 128), (236, 44)]
# uneven h'-halves split at the natural block boundary, so half 0 needs a
# single K=128 matmul per field: (hoff, hwidth) and per-half matmul lists
# of (input_block_idx, K, out_col, M)
HALVES = [(0, 118), (118, 152)]
HBLOCKS = {
    0: [(0, 128, 0, 118)],
    1: [(1, 128, 0, 118), (2, 44, 118, 34)],
}

F16 = mybir.dt.float16
F32 = mybir.dt.float32

FIELDS = ("p", "m", "q", "s")  # x+y, x-y, 2xy, x^2+y^2


def _gauss():
    co = np.arange(WIN, dtype=np.float64) - (WIN // 2)
    g = np.exp(-(co**2) / (2.0 * SIGMA**2))
    return (g / g.sum()).astype(np.float32)


def _band_gain():
    g16 = _gauss().astype(np.float16).astype(np.float64)
    return float(g16.sum())


def _bands():
    g = _gauss()
    bv = np.zeros((128, 118), dtype=np.float32)
    for k in range(128):
        for m in range(118):
            t = k - m
            if 0 <= t <= 10:
                bv[k, m] = g[t]
    bh = np.zeros((128, TW), dtype=np.float32)
    for k in range(128):
        for m in range(TW):
            d3 = k - m - 15
            if d3 % 3 == 0 and -15 <= d3 <= 15:
                bh[k, m] = g[d3 // 3 + 5]
    return bv.astype(np.float16), bh.astype(np.float16)


def _act_raw(eng, out, in_, func, scale=1.0):
    """Emit InstActivation directly (bypasses the bass Reciprocal guard)."""
    ins = [
        eng.lower_ap(in_),
        mybir.ImmediateValue(dtype=mybir.dt.float32, value=0.0),  # bias
        mybir.ImmediateValue(dtype=mybir.dt.float32, value=scale),  # scale
        mybir.ImmediateValue(dtype=mybir.dt.float32, value=0.0),  # alpha
    ]
    return eng.add_instruction(
        mybir.InstActivation(
            name=eng.bass.get_next_instruction_name(),
            func=func,
            ins=ins,
            outs=[eng.lower_ap(out)],
        )
    )


def _pair_view(ap2d, width):
    """[P, >=2*width] -> [P, 2, width] (field-pair view of the first 2w)."""
    return ap2d[:, 0 : 2 * width].rearrange("p (f v) -> p f v", v=width)


def _gpair(t, gw, col, width):
    """group-buffer pair view: [P, 2*gw] -> [P, 2, width] at (col, gw+col)."""
    return t[:, :].rearrange("p (f v) -> p f v", v=gw)[:, :, col : col + width]


def _body(tc, ins, outs):
    nc = tc.nc
    fin = {f: ins[f"f{f}"] for f in FIELDS}
    bv_d, bh_d = ins["bv"], ins["bh"]
    acc_out = outs["acc"]
    ADD, SUB, MUL = (
        mybir.AluOpType.add,
        mybir.AluOpType.subtract,
        mybir.AluOpType.mult,
    )
    ACTF = mybir.ActivationFunctionType
    s = _band_gain()
    SCL = 1.0 / (s * s)  # cancel per-pass f16 band gain at pass1 evac
    KSQ = float(np.sqrt(SC / 2.0))  # Square scale: (KSQ*m)^2 = SC*m^2/2
    C1S = SC * C1
    C2S = SC * C2

    consts = tc.alloc_tile_pool(name="consts", bufs=1)
    inp = tc.alloc_tile_pool(name="inp", bufs=2)
    tvp = tc.alloc_tile_pool(name="tv", bufs=3)
    pvp = tc.alloc_tile_pool(name="pv", bufs=2, space="PSUM")
    p2p = tc.alloc_tile_pool(name="p2", bufs=2, space="PSUM")
    sbp = tc.alloc_tile_pool(name="sb", bufs=2)
    pwp = tc.alloc_tile_pool(name="pw", bufs=2)

    bv_s = consts.tile([128, 118], F16)
    nc.sync.dma_start(out=bv_s, in_=bv_d[:, :])
    bh_s = consts.tile([128, TW], F16)
    nc.sync.dma_start(out=bh_s, in_=bh_d[:, :])
    acc = consts.tile([128, NB], F32)
    nc.vector.memset(acc, 0.0)
    msk_s = consts.tile([128, 2], F32)
    nc.sync.dma_start(out=msk_s, in_=ins["msk"][:, :])

    WGMAX = TW * TGROUP + 30  # 814
    GW = NV * TGROUP  # 2160 columns per field in group buffers

    for grp in range(NB):
        t0 = grp * TGROUP
        nt = min(TGROUP, NT - t0)
        wg0 = TW * t0
        WG = TW * nt + 30
        W2 = NV * nt

        # ---- load input strips: 4 fields x 4 row-blocks ----
        itiles = {}
        for fname in FIELDS:
            for bi, (rs, K) in enumerate(IBLOCKS):
                t = inp.tile(
                    [128, WGMAX], F16, tag=f"in_{fname}_{bi}",
                    name=f"in_{fname}_{bi}_{grp}",
                )
                nc.sync.dma_start(
                    out=t[:K, :WG], in_=fin[fname][rs : rs + K, wg0 : wg0 + WG]
                )
                itiles[(fname, bi)] = t

        # group buffers: field A in cols [0,GW), field B in cols [GW,2GW)
        sbAB = sbp.tile([TW, 2 * GW], F16, tag="sbAB", name=f"sbAB_{grp}")  # A'|B'
        sb34 = sbp.tile([TW, 2 * GW], F16, tag="sb34", name=f"sb34_{grp}")  # S3'|S4'

        for ti in range(nt):
            o = TW * ti
            # tv: pass1 results, [128, 2*NV] per field pair
            tvAB = tvp.tile([128, 2 * NV], F16, tag="tvAB", name=f"tvAB_{grp}_{ti}")
            tvCD = tvp.tile([128, 2 * NV], F16, tag="tvCD", name=f"tvCD_{grp}_{ti}")
            for h, (hoff, hw) in enumerate(HALVES):
                # ---- pass1 (vertical blur), h'-half, fields packed in pairs
                pvm = pvp.tile([128, 304], F32, tag="pvm", name=f"pvm_{grp}_{ti}_{h}")
                pvs = pvp.tile([128, 304], F32, tag="pvs", name=f"pvs_{grp}_{ti}_{h}")
                for pv_t, fpair in ((pvm, ("p", "m")), (pvs, ("q", "s"))):
                    for fi, fname in enumerate(fpair):
                        fc = fi * hw
                        for bi, K, hp, M in HBLOCKS[h]:
                            nc.tensor.matmul(
                                pv_t[:, fc + hp : fc + hp + M],
                                itiles[(fname, bi)][:K, o : o + 128],
                                bv_s[:K, :M],
                                start=True,
                                stop=True,
                            )
                # ---- pass1 evac: one packed-pair instr per engine ----
                nc.scalar.activation(
                    _pair_view(tvAB, NV)[:, :, hoff : hoff + hw],
                    _pair_view(pvm, hw),
                    ACTF.Copy,
                    scale=SCL,
                )
                nc.vector.tensor_scalar(
                    _pair_view(tvCD, NV)[:, :, hoff : hoff + hw],
                    _pair_view(pvs, hw),
                    SCL,
                    None,
                    MUL,
                )
            for h, (hoff, hw) in enumerate(HALVES):
                # ---- pass2 (horizontal blur), h'-half, packed outputs ----
                p2m = p2p.tile([TW, 304], F32, tag="p2m", name=f"p2m_{grp}_{ti}_{h}")
                p2s = p2p.tile([TW, 304], F32, tag="p2s", name=f"p2s_{grp}_{ti}_{h}")
                for p2_t, tv_t in ((p2m, tvAB), (p2s, tvCD)):
                    for fi in (0, 1):
                        nc.tensor.matmul(
                            p2_t[:, fi * hw : (fi + 1) * hw],
                            bh_s,
                            tv_t[:, fi * NV + hoff : fi * NV + hoff + hw],
                            start=True,
                            stop=True,
                        )
                # ---- stage2: fused evac + first pointwise layer ----
                colA = ti * NV + hoff
                nc.scalar.activation(
                    _gpair(sbAB, GW, colA, hw),
                    _pair_view(p2m, hw),
                    ACTF.Square,
                    scale=KSQ,
                )
                nc.vector.tensor_scalar(
                    _gpair(sb34, GW, colA, hw),
                    _pair_view(p2s, hw),
                    SC,
                    C2S,
                    MUL,
                    ADD,
                )

        # ---- group pointwise stage on [98, W2] fp16 ----
        Av = sbAB[:, 0:W2]
        Bv = sbAB[:, GW : GW + W2]
        S3 = sb34[:, 0:W2]
        S4 = sb34[:, GW : GW + W2]
        al0 = pwp.tile([TW, GW], F16, tag="al0", name=f"al0_{grp}")
        u0t = pwp.tile([TW, GW], F16, tag="u0t", name=f"u0t_{grp}")
        alt = pwp.tile([TW, GW], F16, tag="alt", name=f"alt_{grp}")
        u1t = pwp.tile([TW, GW], F16, tag="u1t", name=f"u1t_{grp}")
        rnt = pwp.tile([TW, GW], F16, tag="rnt", name=f"rnt_{grp}")
        rdt = pwp.tile([TW, GW], F16, tag="rdt", name=f"rdt_{grp}")
        numt = pwp.tile([TW, GW], F16, tag="numt", name=f"numt_{grp}")
        dent = pwp.tile([TW, GW], F16, tag="dent", name=f"dent_{grp}")
        rect = pwp.tile([TW, GW], F16, tag="rect", name=f"rect_{grp}")

        # al0 = A' - B' = SC * mux*muy              (DVE)
        nc.vector.tensor_tensor(al0[:, :W2], Av, Bv, SUB)
        # u0 = A' + B' = SC/2 * (mux^2 + muy^2)     (GPSIMD)
        nc.gpsimd.tensor_tensor(u0t[:, :W2], Av, Bv, ADD)
        # rn = S3' - 2*al0 ... careful with factors, see note below
        # S3' = SC*(2*blur(xy)) + SC*C2 ; al0 = SC*(2 mux muy)/1 ?
        # al0 = (SC/2)(m1^2 - m2^2)/1 = SC/2 * 4 mux muy = 2*SC*mux*muy
        # => rn = S3' - al0 = SC*(2 sigxy + C2)     (DVE)
        nc.vector.tensor_tensor(rnt[:, :W2], S3, al0[:, :W2], SUB)
        # al = al0 + SC*C1 = SC*(2 mux muy + C1)    (DVE TS)
        nc.vector.tensor_scalar(alt[:, :W2], al0[:, :W2], 1.0, C1S, MUL, ADD)
        # rd = S4' - u0 = SC*(sigxx + sigyy + C2)   (GPSIMD, after u0)
        nc.gpsimd.tensor_tensor(rdt[:, :W2], S4, u0t[:, :W2], SUB)
        # u1 = u0 + SC*C1 = SC*(mux^2+muy^2 + C1)   (DVE TS)
        nc.vector.tensor_scalar(u1t[:, :W2], u0t[:, :W2], 1.0, C1S, MUL, ADD)
        # num = al * rn                              (DVE)
        nc.vector.tensor_tensor(numt[:, :W2], alt[:, :W2], rnt[:, :W2], MUL)
        # den = u1 * rd                              (DVE)
        nc.vector.tensor_tensor(dent[:, :W2], u1t[:, :W2], rdt[:, :W2], MUL)
        if t0 + nt == NT:
            # last output tile (t=117): only 54 valid wc' partitions.
            # num <- num*m ; den <- den*m + (1-m)  (m: 1 valid / 0 invalid)
            iv0 = NV * (NT - 1 - t0)
            nc.vector.tensor_scalar(
                numt[:TW, iv0 : iv0 + NV], numt[:TW, iv0 : iv0 + NV],
                msk_s[:TW, 0:1], 0.0, MUL, ADD,
            )
            nc.vector.tensor_scalar(
                dent[:TW, iv0 : iv0 + NV], dent[:TW, iv0 : iv0 + NV],
                msk_s[:TW, 0:1], msk_s[:TW, 1:2], MUL, ADD,
            )
        # rec = 1/den                                (ACT Reciprocal, raw)
        _act_raw(nc.scalar, rect[:, :W2], dent[:, :W2], ACTF.Reciprocal)
        # ssim = num * rec; acc[:, grp] += sum      (DVE TT + TS accum)
        nc.vector.tensor_tensor(numt[:, :W2], numt[:, :W2], rect[:, :W2], MUL)
        nc.vector.tensor_scalar(
            al0[:, :W2], numt[:, :W2], 1.0, 0.0, MUL, ADD,
            accum_out=acc[:TW, grp : grp + 1],
        )

    nc.sync.dma_start(out=acc_out[:, :], in_=acc)

    for p in (pwp, sbp, p2p, pvp, tvp, inp, consts):
        p.release()


_CACHE = {}


def _get_compiled():
    if "nc" in _CACHE:
        return _CACHE["nc"], _CACHE["aps"]
    nc = bacc.Bacc("TRN2", target_bir_lowering=False, debug=False, num_devices=NCORES)
    ins = {}
    for f in FIELDS:
        ins[f"f{f}"] = nc.dram_tensor(
            f"f{f}", [HSLAB, WPAD], F16, kind="ExternalInput"
        ).ap()
    ins["bv"] = nc.dram_tensor("bv", [128, 118], F16, kind="ExternalInput").ap()
    ins["bh"] = nc.dram_tensor("bh", [128, TW], F16, kind="ExternalInput").ap()
    ins["msk"] = nc.dram_tensor("msk", [128, 2], F32, kind="ExternalInput").ap()
    outs = {"acc": nc.dram_tensor("acc", [128, NB], F32, kind="ExternalOutput").ap()}
    with tile.TileContext(nc) as tc:
        _body(tc, ins, outs)
    nc.compile()
    _CACHE["nc"] = nc
    _CACHE["aps"] = (ins, outs)
    return nc, (ins, outs)


LAST_RES = None


def kernel(X, Y, _trace=False, _trace_kwargs=None):
    global LAST_RES
    X = np.asarray(X, dtype=np.float32).reshape(H, WC)
    Y = np.asarray(Y, dtype=np.float32).reshape(H, WC)

    bv, bh = _bands()
    nvalid = WC - TW * (NT - 1)  # 54
    msk = np.zeros((128, 2), dtype=np.float32)
    msk[:nvalid, 0] = 1.0
    msk[:, 1] = 1.0 - msk[:, 0]
    fields = {
        "fp": X + Y,
        "fm": X - Y,
        "fq": 2.0 * (X * Y),
        "fs": X * X + Y * Y,
    }
    # pad rows (5 top/bottom) and wc (15 left, to WPAD right), cast fp16
    padded = {}
    for k, a in fields.items():
        p = np.zeros((H + 10, WPAD), dtype=np.float16)
        p[5 : 5 + H, PADL : PADL + WC] = a.astype(np.float16)
        padded[k] = p

    in_maps = []
    for c in range(NCORES):
        m = {
            k: np.ascontiguousarray(p[HOUT * c : HOUT * c + HSLAB])
            for k, p in padded.items()
        }
        m["bv"] = bv
        m["bh"] = bh
        m["msk"] = msk
        in_maps.append(m)

    nc, _ = _get_compiled()
    res = run_bass_kernel_spmd(
        nc, in_maps, core_ids=list(range(NCORES)),
        trace=_trace, **(_trace_kwargs or {}),
    )
    LAST_RES = res
    total = 0.0
    for r in res.results:
        total += float(np.asarray(r["acc"])[:TW, :].astype(np.float64).sum())
    # acc holds SC*num/(SC*den) = ssim, summed over all pixels
    loss = 1.0 - total / (H * W * C)
    return np.float32(loss)


# revision 4
# speedup vs baseline: 1.4491x; 1.1185x over previous
"""DSSIM loss kernel for Trainium2, 8 NeuronCores — v2.

Strategy vs v1 baseline (705us), per trace analysis:
  - Host fields {x+y, x-y, 2xy, x^2+y^2}: the sigma algebra becomes linear
    in the blurred fields, so the pointwise stage is 7 TT + 2 TS + 1 accum
    on DVE/GPSIMD (v1 used scalar_tensor_tensor at ~4us each).
  - ACT uses only Copy/Square/Reciprocal — all live in the single
    `reciprocal_and_small` activation-table set: removes 31 ACT_TABLE_LOADs
    (47us + serialization) and does the division in one pass (v1: Ln+Exp).
    Reciprocal is emitted raw (bass guards it for accuracy; DSSIM slack is
    huge: ssim~0.007, so 1% recip error moves the loss ~7e-5).
  - All pointwise math scaled by 8 (folded into free scale slots) so 1/den
    stays < ~250 and fits fp16 comfortably.
  - PSUM: both passes tiled in h'-halves (135) with two fields packed per
    2KB bank; pv and p2 pools double-buffered (4+4 banks) so the tensor
    engine never waits on evacuations (v1: 529us MATMUL semaphore wait,
    single-buffered PSUM).
  - pass1 evac: ACT Copy, one instr per packed field-pair.
  - pass2 "evac" fused with math: Square(psum) on ACT for the mean fields,
    one 2-op tensor_scalar from PSUM on DVE for the second-moment fields.
  - Final sum via tensor_scalar accum_out into per-group acc columns.
"""

import sys

sys.path.insert(0, "/opt/trn_rl_repo")

import numpy as np

import concourse.bass as bass
import concourse.bacc as bacc
import concourse.tile as tile
from concourse import mybir
from concourse.bass_utils import run_bass_kernel_spmd

# ---------------- problem geometry (hardcoded) ----------------
H, W, C = 2160, 3840, 3
WC = W * C  # 11520
NCORES = 8
HOUT = H // NCORES  # 270
HSLAB = HOUT + 10  # 280 input rows per core (5 halo each side)
PADL = 15  # left zero pad in wc (= 5 pixels * 3 channels)
TW = 98  # output tile width in wc' (128 - 2*15)
NT = (WC + TW - 1) // TW  # 118 tiles; last tile has 54 valid columns
WPAD = TW * (NT - 1) + 128  # 11594
WPAD = ((WPAD + 7) // 8) * 8  # 11600
TGROUP = 8
NB = (NT + TGROUP - 1) // TGROUP  # 15 groups
NV = HOUT  # 270 (h' extent)
HNV = NV // 2  # 135 (h' half)
WIN, SIGMA = 11, 1.5
C1, C2 = 0.01**2, 0.03**2
SC = 8.0  # range scale folded into num and den (cancels in the ratio)

# input row-blocks (baseline 3-block layout):
IBLOCKS = [(0, 128), (118, 128), (236, 44)]
# uneven h'-halves split at the natural block boundary, so half 0 needs a
# single K=128 matmul per field: (hoff, hwidth) and per-half matmul lists
# of (input_block_idx, K, out_col, M)
HALVES = [(0, 118), (118, 152)]
HBLOCKS = {
    0: [(0, 128, 0, 118)],
    1: [(1, 128, 0, 118), (2, 44, 118, 34)],
}

F16 = mybir.dt.float16
F32 = mybir.dt.float32

FIELDS = ("p", "m", "q", "s")  # x+y, x-y, 2xy, x^2+y^2


def _gauss():
    co = np.arange(WIN, dtype=np.float64) - (WIN // 2)
    g = np.exp(-(co**2) / (2.0 * SIGMA**2))
    return (g / g.sum()).astype(np.float32)


def _band_gain():
    g16 = _gauss().astype(np.float16).astype(np.float64)
    return float(g16.sum())


def _bands():
    g = _gauss()
    bv = np.zeros((128, 118), dtype=np.float32)
    for k in range(128):
        for m in range(118):
            t = k - m
            if 0 <= t <= 10:
                bv[k, m] = g[t]
    bh = np.zeros((128, TW), dtype=np.float32)
    for k in range(128):
        for m in range(TW):
            d3 = k - m - 15
            if d3 % 3 == 0 and -15 <= d3 <= 15:
                bh[k, m] = g[d3 // 3 + 5]
    return bv.astype(np.float16), bh.astype(np.float16)


def _act_raw(eng, out, in_, func, scale=1.0):
    """Emit InstActivation directly (bypasses the bass Reciprocal guard)."""
    ins = [
        eng.lower_ap(in_),
        mybir.ImmediateValue(dtype=mybir.dt.float32, value=0.0),  # bias
        mybir.ImmediateValue(dtype=mybir.dt.float32, value=scale),  # scale
        mybir.ImmediateValue(dtype=mybir.dt.float32, value=0.0),  # alpha
    ]
    return eng.add_instruction(
        mybir.InstActivation(
            name=eng.bass.get_next_instruction_name(),
            func=func,
            ins=ins,
            outs=[eng.lower_ap(out)],
        )
    )


def _pair_view(ap2d, width):
    """[P, >=2*width] -> [P, 2, width] (field-pair view of the first 2w)."""
    return ap2d[:, 0 : 2 * width].rearrange("p (f v) -> p f v", v=width)


def _gpair(t, gw, col, width):
    """group-buffer pair view: [P, 2*gw] -> [P, 2, width] at (col, gw+col)."""
    return t[:, :].rearrange("p (f v) -> p f v", v=gw)[:, :, col : col + width]


def _body(tc, ins, outs):
    nc = tc.nc
    fin = {f: ins[f"f{f}"] for f in FIELDS}
    bv_d, bh_d = ins["bv"], ins["bh"]
    acc_out = outs["acc"]
    ADD, SUB, MUL = (
        mybir.AluOpType.add,
        mybir.AluOpType.subtract,
        mybir.AluOpType.mult,
    )
    ACTF = mybir.ActivationFunctionType
    s = _band_gain()
    SCL = 1.0 / (s * s)  # cancel per-pass f16 band gain at pass1 evac
    KSQ = float(np.sqrt(SC / 2.0))  # Square scale: (KSQ*m)^2 = SC*m^2/2
    C1S = SC * C1
    C2S = SC * C2

    consts = tc.alloc_tile_pool(name="consts", bufs=1)
    inp = tc.alloc_tile_pool(name="inp", bufs=2)
    tvp = tc.alloc_tile_pool(name="tv", bufs=3)
    pvp = tc.alloc_tile_pool(name="pv", bufs=2, space="PSUM")
    p2p = tc.alloc_tile_pool(name="p2", bufs=1, space="PSUM")
    sbp = tc.alloc_tile_pool(name="sb", bufs=2)
    pwp = tc.alloc_tile_pool(name="pw", bufs=2)

    bv_s = consts.tile([128, 118], F16)
    nc.sync.dma_start(out=bv_s, in_=bv_d[:, :])
    bh_s = consts.tile([128, TW], F16)
    nc.sync.dma_start(out=bh_s, in_=bh_d[:, :])
    acc = consts.tile([128, NB], F32)
    nc.vector.memset(acc, 0.0)
    msk_s = consts.tile([128, 2], F32)
    nc.sync.dma_start(out=msk_s, in_=ins["msk"][:, :])

    WGMAX = TW * TGROUP + 30  # 814
    GW = NV * TGROUP  # 2160 columns per field in group buffers

    for grp in range(NB):
        t0 = grp * TGROUP
        nt = min(TGROUP, NT - t0)
        wg0 = TW * t0
        WG = TW * nt + 30
        W2 = NV * nt

        # ---- load input strips: 4 fields x 4 row-blocks ----
        itiles = {}
        for fname in FIELDS:
            for bi, (rs, K) in enumerate(IBLOCKS):
                t = inp.tile(
                    [128, WGMAX], F16, tag=f"in_{fname}_{bi}",
                    name=f"in_{fname}_{bi}_{grp}",
                )
                nc.sync.dma_start(
                    out=t[:K, :WG], in_=fin[fname][rs : rs + K, wg0 : wg0 + WG]
                )
                itiles[(fname, bi)] = t

        # group buffers: field A in cols [0,GW), field B in cols [GW,2GW)
        sbAB = sbp.tile([TW, 2 * GW], F16, tag="sbAB", name=f"sbAB_{grp}")  # A'|B'
        sb34 = sbp.tile([TW, 2 * GW], F16, tag="sb34", name=f"sb34_{grp}")  # S3'|S4'

        for ti in range(nt):
            o = TW * ti
            # tv: pass1 results, [128, 2*NV] per field pair
            tvAB = tvp.tile([128, 2 * NV], F16, tag="tvAB", name=f"tvAB_{grp}_{ti}")
            tvCD = tvp.tile([128, 2 * NV], F16, tag="tvCD", name=f"tvCD_{grp}_{ti}")
            for h, (hoff, hw) in enumerate(HALVES):
                # ---- pass1 (vertical blur), h'-half, fields packed in pairs
                pvm = pvp.tile([128, 304], F32, tag="pvm", name=f"pvm_{grp}_{ti}_{h}")
                pvs = pvp.tile([128, 304], F32, tag="pvs", name=f"pvs_{grp}_{ti}_{h}")
                for pv_t, fpair in ((pvm, ("p", "m")), (pvs, ("q", "s"))):
                    for fi, fname in enumerate(fpair):
                        fc = fi * hw
                        for bi, K, hp, M in HBLOCKS[h]:
                            nc.tensor.matmul(
                                pv_t[:, fc + hp : fc + hp + M],
                                itiles[(fname, bi)][:K, o : o + 128],
                                bv_s[:K, :M],
                                start=True,
                                stop=True,
                            )
                # ---- pass1 evac: one packed-pair instr per engine ----
                nc.scalar.activation(
                    _pair_view(tvAB, NV)[:, :, hoff : hoff + hw],
                    _pair_view(pvm, hw),
                    ACTF.Copy,
                    scale=SCL,
                )
                nc.vector.tensor_scalar(
                    _pair_view(tvCD, NV)[:, :, hoff : hoff + hw],
                    _pair_view(pvs, hw),
                    SCL,
                    None,
                    MUL,
                )
            # ---- pass2 (horizontal blur): full-height, one MM per field
            p2t = {}
            for fi, fname in enumerate(FIELDS):
                pt = p2p.tile([TW, NV], F32, tag=f"p2{fname}", name=f"p2{fname}_{grp}_{ti}")
                tv_t = tvAB if fi < 2 else tvCD
                nc.tensor.matmul(
                    pt,
                    bh_s,
                    tv_t[:, (fi % 2) * NV : (fi % 2) * NV + NV],
                    start=True,
                    stop=True,
                )
                p2t[fname] = pt
            # ---- stage2: fused evac + first pointwise layer ----
            colA = ti * NV
            nc.scalar.activation(
                sbAB[:, colA : colA + NV], p2t["p"], ACTF.Square, scale=KSQ
            )
            nc.scalar.activation(
                sbAB[:, GW + colA : GW + colA + NV], p2t["m"], ACTF.Square, scale=KSQ
            )
            nc.vector.tensor_scalar(
                sb34[:, colA : colA + NV], p2t["q"], SC, C2S, MUL, ADD
            )
            nc.vector.tensor_scalar(
                sb34[:, GW + colA : GW + colA + NV], p2t["s"], SC, C2S, MUL, ADD
            )

        # ---- group pointwise stage on [98, W2] fp16 ----
        Av = sbAB[:, 0:W2]
        Bv = sbAB[:, GW : GW + W2]
        S3 = sb34[:, 0:W2]
        S4 = sb34[:, GW : GW + W2]
        al0 = pwp.tile([TW, GW], F16, tag="al0", name=f"al0_{grp}")
        u0t = pwp.tile([TW, GW], F16, tag="u0t", name=f"u0t_{grp}")
        alt = pwp.tile([TW, GW], F16, tag="alt", name=f"alt_{grp}")
        u1t = pwp.tile([TW, GW], F16, tag="u1t", name=f"u1t_{grp}")
        rnt = pwp.tile([TW, GW], F16, tag="rnt", name=f"rnt_{grp}")
        rdt = pwp.tile([TW, GW], F16, tag="rdt", name=f"rdt_{grp}")
        numt = pwp.tile([TW, GW], F16, tag="numt", name=f"numt_{grp}")
        dent = pwp.tile([TW, GW], F16, tag="dent", name=f"dent_{grp}")
        rect = pwp.tile([TW, GW], F16, tag="rect", name=f"rect_{grp}")

        # al0 = A' - B' = SC * mux*muy              (DVE)
        nc.vector.tensor_tensor(al0[:, :W2], Av, Bv, SUB)
        # u0 = A' + B' = SC/2 * (mux^2 + muy^2)     (GPSIMD)
        nc.gpsimd.tensor_tensor(u0t[:, :W2], Av, Bv, ADD)
        # rn = S3' - 2*al0 ... careful with factors, see note below
        # S3' = SC*(2*blur(xy)) + SC*C2 ; al0 = SC*(2 mux muy)/1 ?
        # al0 = (SC/2)(m1^2 - m2^2)/1 = SC/2 * 4 mux muy = 2*SC*mux*muy
        # => rn = S3' - al0 = SC*(2 sigxy + C2)     (DVE)
        nc.vector.tensor_tensor(rnt[:, :W2], S3, al0[:, :W2], SUB)
        # al = al0 + SC*C1 = SC*(2 mux muy + C1)    (DVE TS)
        nc.vector.tensor_scalar(alt[:, :W2], al0[:, :W2], 1.0, C1S, MUL, ADD)
        # rd = S4' - u0 = SC*(sigxx + sigyy + C2)   (GPSIMD, after u0)
        nc.gpsimd.tensor_tensor(rdt[:, :W2], S4, u0t[:, :W2], SUB)
        # u1 = u0 + SC*C1 = SC*(mux^2+muy^2 + C1)   (DVE TS)
        nc.vector.tensor_scalar(u1t[:, :W2], u0t[:, :W2], 1.0, C1S, MUL, ADD)
        # num = al * rn                              (DVE)
        nc.vector.tensor_tensor(numt[:, :W2], alt[:, :W2], rnt[:, :W2], MUL)
        # den = u1 * rd                              (DVE)
        nc.vector.tensor_tensor(dent[:, :W2], u1t[:, :W2], rdt[:, :W2], MUL)
        if t0 + nt == NT:
            # last output tile (t=117): only 54 valid wc' partitions.
            # num <- num*m ; den <- den*m + (1-m)  (m: 1 valid / 0 invalid)
            iv0 = NV * (NT - 1 - t0)
            nc.vector.tensor_scalar(
                numt[:TW, iv0 : iv0 + NV], numt[:TW, iv0 : iv0 + NV],
                msk_s[:TW, 0:1], 0.0, MUL, ADD,
            )
            nc.vector.tensor_scalar(
                dent[:TW, iv0 : iv0 + NV], dent[:TW, iv0 : iv0 + NV],
                msk_s[:TW, 0:1], msk_s[:TW, 1:2], MUL, ADD,
            )
        # rec = 1/den                                (ACT Reciprocal, raw)
        _act_raw(nc.scalar, rect[:, :W2], dent[:, :W2], ACTF.Reciprocal)
        # ssim = num * rec; acc[:, grp] += sum      (DVE TT + TS accum)
        nc.vector.tensor_tensor(numt[:, :W2], numt[:, :W2], rect[:, :W2], MUL)
        nc.vector.tensor_scalar(
            al0[:, :W2], numt[:, :W2], 1.0, 0.0, MUL, ADD,
            accum_out=acc[:TW, grp : grp + 1],
        )

    nc.sync.dma_start(out=acc_out[:, :], in_=acc)

    for p in (pwp, sbp, p2p, pvp, tvp, inp, consts):
        p.release()


_CACHE = {}


def _get_compiled():
    if "nc" in _CACHE:
        return _CACHE["nc"], _CACHE["aps"]
    nc = bacc.Bacc("TRN2", target_bir_lowering=False, debug=False, num_devices=NCORES)
    ins = {}
    for f in FIELDS:
        ins[f"f{f}"] = nc.dram_tensor(
            f"f{f}", [HSLAB, WPAD], F16, kind="ExternalInput"
        ).ap()
    ins["bv"] = nc.dram_tensor("bv", [128, 118], F16, kind="ExternalInput").ap()
    ins["bh"] = nc.dram_tensor("bh", [128, TW], F16, kind="ExternalInput").ap()
    ins["msk"] = nc.dram_tensor("msk", [128, 2], F32, kind="ExternalInput").ap()
    outs = {"acc": nc.dram_tensor("acc", [128, NB], F32, kind="ExternalOutput").ap()}
    with tile.TileContext(nc) as tc:
        _body(tc, ins, outs)
    nc.compile()
    _CACHE["nc"] = nc
    _CACHE["aps"] = (ins, outs)
    return nc, (ins, outs)


LAST_RES = None


def kernel(X, Y, _trace=False, _trace_kwargs=None):
    global LAST_RES
    X = np.asarray(X, dtype=np.float32).reshape(H, WC)
    Y = np.asarray(Y, dtype=np.float32).reshape(H, WC)

    bv, bh = _bands()
    nvalid = WC - TW * (NT - 1)  # 54
    msk = np.zeros((128, 2), dtype=np.float32)
    msk[:nvalid, 0] = 1.0
    msk[:, 1] = 1.0 - msk[:, 0]
    fields = {
        "fp": X + Y,
        "fm": X - Y,
        "fq": 2.0 * (X * Y),
        "fs": X * X + Y * Y,
    }
    # pad rows (5 top/bottom) and wc (15 left, to WPAD right), cast fp16
    padded = {}
    for k, a in fields.items():
        p = np.zeros((H + 10, WPAD), dtype=np.float16)
        p[5 : 5 + H, PADL : PADL + WC] = a.astype(np.float16)
        padded[k] = p

    in_maps = []
    for c in range(NCORES):
        m = {
            k: np.ascontiguousarray(p[HOUT * c : HOUT * c + HSLAB])
            for k, p in padded.items()
        }
        m["bv"] = bv
        m["bh"] = bh
        m["msk"] = msk
        in_maps.append(m)

    nc, _ = _get_compiled()
    res = run_bass_kernel_spmd(
        nc, in_maps, core_ids=list(range(NCORES)),
        trace=_trace, **(_trace_kwargs or {}),
    )
    LAST_RES = res
    total = 0.0
    for r in res.results:
        total += float(np.asarray(r["acc"])[:TW, :].astype(np.float64).sum())
    # acc holds SC*num/(SC*den) = ssim, summed over all pixels
    loss = 1.0 - total / (H * W * C)
    return np.float32(loss)


# revision 5
# speedup vs baseline: 1.4561x; 1.0048x over previous
"""DSSIM loss kernel for Trainium2, 8 NeuronCores — v2.

Strategy vs v1 baseline (705us), per trace analysis:
  - Host fields {x+y, x-y, 2xy, x^2+y^2}: the sigma algebra becomes linear
    in the blurred fields, so the pointwise stage is 7 TT + 2 TS + 1 accum
    on DVE/GPSIMD (v1 used scalar_tensor_tensor at ~4us each).
  - ACT uses only Copy/Square/Reciprocal — all live in the single
    `reciprocal_and_small` activation-table set: removes 31 ACT_TABLE_LOADs
    (47us + serialization) and does the division in one pass (v1: Ln+Exp).
    Reciprocal is emitted raw (bass guards it for accuracy; DSSIM slack is
    huge: ssim~0.007, so 1% recip error moves the loss ~7e-5).
  - All pointwise math scaled by 8 (folded into free scale slots) so 1/den
    stays < ~250 and fits fp16 comfortably.
  - PSUM: both passes tiled in h'-halves (135) with two fields packed per
    2KB bank; pv and p2 pools double-buffered (4+4 banks) so the tensor
    engine never waits on evacuations (v1: 529us MATMUL semaphore wait,
    single-buffered PSUM).
  - pass1 evac: ACT Copy, one instr per packed field-pair.
  - pass2 "evac" fused with math: Square(psum) on ACT for the mean fields,
    one 2-op tensor_scalar from PSUM on DVE for the second-moment fields.
  - Final sum via tensor_scalar accum_out into per-group acc columns.
"""

import sys

sys.path.insert(0, "/opt/trn_rl_repo")

import numpy as np

import concourse.bass as bass
import concourse.bacc as bacc
import concourse.tile as tile
from concourse import mybir
from concourse.bass_utils import run_bass_kernel_spmd

# ---------------- problem geometry (hardcoded) ----------------
H, W, C = 2160, 3840, 3
WC = W * C  # 11520
NCORES = 8
HOUT = H // NCORES  # 270
HSLAB = HOUT + 10  # 280 input rows per core (5 halo each side)
PADL = 15  # left zero pad in wc (= 5 pixels * 3 channels)
TW = 98  # output tile width in wc' (128 - 2*15)
NT = (WC + TW - 1) // TW  # 118 tiles; last tile has 54 valid columns
WPAD = TW * (NT - 1) + 128  # 11594
WPAD = ((WPAD + 7) // 8) * 8  # 11600
TGROUP = 8
NB = (NT + TGROUP - 1) // TGROUP  # 15 groups
NV = HOUT  # 270 (h' extent)
HNV = NV // 2  # 135 (h' half)
WIN, SIGMA = 11, 1.5
C1, C2 = 0.01**2, 0.03**2
SC = 8.0  # range scale folded into num and den (cancels in the ratio)

# input row-blocks (baseline 3-block layout):
IBLOCKS = [(0, 128), (118, 128), (236, 44)]
# uneven h'-halves split at the natural block boundary, so half 0 needs a
# single K=128 matmul per field: (hoff, hwidth) and per-half matmul lists
# of (input_block_idx, K, out_col, M)
HALVES = [(0, 118), (118, 152)]
HBLOCKS = {
    0: [(0, 128, 0, 118)],
    1: [(1, 128, 0, 118), (2, 44, 118, 34)],
}

F16 = mybir.dt.float16
F32 = mybir.dt.float32

FIELDS = ("p", "m", "q", "s")  # x+y, x-y, 2xy, x^2+y^2


def _gauss():
    co = np.arange(WIN, dtype=np.float64) - (WIN // 2)
    g = np.exp(-(co**2) / (2.0 * SIGMA**2))
    return (g / g.sum()).astype(np.float32)


def _band_gain():
    g16 = _gauss().astype(np.float16).astype(np.float64)
    return float(g16.sum())


def _bands():
    g = _gauss()
    bv = np.zeros((128, 118), dtype=np.float32)
    for k in range(128):
        for m in range(118):
            t = k - m
            if 0 <= t <= 10:
                bv[k, m] = g[t]
    bh = np.zeros((128, TW), dtype=np.float32)
    for k in range(128):
        for m in range(TW):
            d3 = k - m - 15
            if d3 % 3 == 0 and -15 <= d3 <= 15:
                bh[k, m] = g[d3 // 3 + 5]
    return bv.astype(np.float16), bh.astype(np.float16)


def _act_raw(eng, out, in_, func, scale=1.0):
    """Emit InstActivation directly (bypasses the bass Reciprocal guard)."""
    ins = [
        eng.lower_ap(in_),
        mybir.ImmediateValue(dtype=mybir.dt.float32, value=0.0),  # bias
        mybir.ImmediateValue(dtype=mybir.dt.float32, value=scale),  # scale
        mybir.ImmediateValue(dtype=mybir.dt.float32, value=0.0),  # alpha
    ]
    return eng.add_instruction(
        mybir.InstActivation(
            name=eng.bass.get_next_instruction_name(),
            func=func,
            ins=ins,
            outs=[eng.lower_ap(out)],
        )
    )


def _pair_view(ap2d, width):
    """[P, >=2*width] -> [P, 2, width] (field-pair view of the first 2w)."""
    return ap2d[:, 0 : 2 * width].rearrange("p (f v) -> p f v", v=width)


def _gpair(t, gw, col, width):
    """group-buffer pair view: [P, 2*gw] -> [P, 2, width] at (col, gw+col)."""
    return t[:, :].rearrange("p (f v) -> p f v", v=gw)[:, :, col : col + width]


def _body(tc, ins, outs):
    nc = tc.nc
    fin = {f: ins[f"f{f}"] for f in FIELDS}
    bv_d, bh_d = ins["bv"], ins["bh"]
    acc_out = outs["acc"]
    ADD, SUB, MUL = (
        mybir.AluOpType.add,
        mybir.AluOpType.subtract,
        mybir.AluOpType.mult,
    )
    ACTF = mybir.ActivationFunctionType
    s = _band_gain()
    SCL = 1.0 / (s * s)  # cancel per-pass f16 band gain at pass1 evac
    KSQ = float(np.sqrt(SC / 2.0))  # Square scale: (KSQ*m)^2 = SC*m^2/2
    C1S = SC * C1
    C2S = SC * C2

    consts = tc.alloc_tile_pool(name="consts", bufs=1)
    inp = tc.alloc_tile_pool(name="inp", bufs=3)
    tvp = tc.alloc_tile_pool(name="tv", bufs=3)
    pvp = tc.alloc_tile_pool(name="pv", bufs=2, space="PSUM")
    p2p = tc.alloc_tile_pool(name="p2", bufs=1, space="PSUM")
    sbp = tc.alloc_tile_pool(name="sb", bufs=2)
    pwp = tc.alloc_tile_pool(name="pw", bufs=2)

    bv_s = consts.tile([128, 118], F16)
    nc.sync.dma_start(out=bv_s, in_=bv_d[:, :])
    bh_s = consts.tile([128, TW], F16)
    nc.sync.dma_start(out=bh_s, in_=bh_d[:, :])
    acc = consts.tile([128, NB], F32)
    nc.vector.memset(acc, 0.0)
    msk_s = consts.tile([128, 2], F32)
    nc.sync.dma_start(out=msk_s, in_=ins["msk"][:, :])

    WGMAX = TW * TGROUP + 30  # 814
    GW = NV * TGROUP  # 2160 columns per field in group buffers

    for grp in range(NB):
        t0 = grp * TGROUP
        nt = min(TGROUP, NT - t0)
        wg0 = TW * t0
        WG = TW * nt + 30
        W2 = NV * nt

        # ---- load input strips: 4 fields x 4 row-blocks ----
        itiles = {}
        for fname in FIELDS:
            for bi, (rs, K) in enumerate(IBLOCKS):
                t = inp.tile(
                    [128, WGMAX], F16, tag=f"in_{fname}_{bi}",
                    name=f"in_{fname}_{bi}_{grp}",
                )
                nc.sync.dma_start(
                    out=t[:K, :WG], in_=fin[fname][rs : rs + K, wg0 : wg0 + WG]
                )
                itiles[(fname, bi)] = t

        # group buffers: field A in cols [0,GW), field B in cols [GW,2GW)
        sbAB = sbp.tile([TW, 2 * GW], F16, tag="sbAB", name=f"sbAB_{grp}")  # A'|B'
        sb34 = sbp.tile([TW, 2 * GW], F16, tag="sb34", name=f"sb34_{grp}")  # S3'|S4'

        for ti in range(nt):
            o = TW * ti
            # tv: pass1 results, [128, 2*NV] per field pair
            tvAB = tvp.tile([128, 2 * NV], F16, tag="tvAB", name=f"tvAB_{grp}_{ti}")
            tvCD = tvp.tile([128, 2 * NV], F16, tag="tvCD", name=f"tvCD_{grp}_{ti}")
            for h, (hoff, hw) in enumerate(HALVES):
                # ---- pass1 (vertical blur), h'-half, fields packed in pairs
                pvm = pvp.tile([128, 304], F32, tag="pvm", name=f"pvm_{grp}_{ti}_{h}")
                pvs = pvp.tile([128, 304], F32, tag="pvs", name=f"pvs_{grp}_{ti}_{h}")
                for pv_t, fpair in ((pvm, ("p", "m")), (pvs, ("q", "s"))):
                    for fi, fname in enumerate(fpair):
                        fc = fi * hw
                        for bi, K, hp, M in HBLOCKS[h]:
                            nc.tensor.matmul(
                                pv_t[:, fc + hp : fc + hp + M],
                                itiles[(fname, bi)][:K, o : o + 128],
                                bv_s[:K, :M],
                                start=True,
                                stop=True,
                            )
                # ---- pass1 evac: one packed-pair instr per engine ----
                nc.scalar.activation(
                    _pair_view(tvAB, NV)[:, :, hoff : hoff + hw],
                    _pair_view(pvm, hw),
                    ACTF.Copy,
                    scale=SCL,
                )
                nc.vector.tensor_scalar(
                    _pair_view(tvCD, NV)[:, :, hoff : hoff + hw],
                    _pair_view(pvs, hw),
                    SCL,
                    None,
                    MUL,
                )
            # ---- pass2 (horizontal blur): full-height, one MM per field
            p2t = {}
            for fi, fname in enumerate(FIELDS):
                pt = p2p.tile([TW, NV], F32, tag=f"p2{fname}", name=f"p2{fname}_{grp}_{ti}")
                tv_t = tvAB if fi < 2 else tvCD
                nc.tensor.matmul(
                    pt,
                    bh_s,
                    tv_t[:, (fi % 2) * NV : (fi % 2) * NV + NV],
                    start=True,
                    stop=True,
                )
                p2t[fname] = pt
            # ---- stage2: fused evac + first pointwise layer ----
            colA = ti * NV
            nc.scalar.activation(
                sbAB[:, colA : colA + NV], p2t["p"], ACTF.Square, scale=KSQ
            )
            nc.scalar.activation(
                sbAB[:, GW + colA : GW + colA + NV], p2t["m"], ACTF.Square, scale=KSQ
            )
            nc.vector.tensor_scalar(
                sb34[:, colA : colA + NV], p2t["q"], SC, C2S, MUL, ADD
            )
            nc.vector.tensor_scalar(
                sb34[:, GW + colA : GW + colA + NV], p2t["s"], SC, C2S, MUL, ADD
            )

        # ---- group pointwise stage on [98, W2] fp16 ----
        Av = sbAB[:, 0:W2]
        Bv = sbAB[:, GW : GW + W2]
        S3 = sb34[:, 0:W2]
        S4 = sb34[:, GW : GW + W2]
        al0 = pwp.tile([TW, GW], F16, tag="al0", name=f"al0_{grp}")
        u0t = pwp.tile([TW, GW], F16, tag="u0t", name=f"u0t_{grp}")
        alt = pwp.tile([TW, GW], F16, tag="alt", name=f"alt_{grp}")
        u1t = pwp.tile([TW, GW], F16, tag="u1t", name=f"u1t_{grp}")
        rnt = pwp.tile([TW, GW], F16, tag="rnt", name=f"rnt_{grp}")
        rdt = pwp.tile([TW, GW], F16, tag="rdt", name=f"rdt_{grp}")
        numt = pwp.tile([TW, GW], F16, tag="numt", name=f"numt_{grp}")
        dent = pwp.tile([TW, GW], F16, tag="dent", name=f"dent_{grp}")
        rect = pwp.tile([TW, GW], F16, tag="rect", name=f"rect_{grp}")

        # al0 = A' - B' = SC * mux*muy              (DVE)
        nc.vector.tensor_tensor(al0[:, :W2], Av, Bv, SUB)
        # u0 = A' + B' = SC/2 * (mux^2 + muy^2)     (GPSIMD)
        nc.gpsimd.tensor_tensor(u0t[:, :W2], Av, Bv, ADD)
        # rn = S3' - 2*al0 ... careful with factors, see note below
        # S3' = SC*(2*blur(xy)) + SC*C2 ; al0 = SC*(2 mux muy)/1 ?
        # al0 = (SC/2)(m1^2 - m2^2)/1 = SC/2 * 4 mux muy = 2*SC*mux*muy
        # => rn = S3' - al0 = SC*(2 sigxy + C2)     (DVE)
        nc.vector.tensor_tensor(rnt[:, :W2], S3, al0[:, :W2], SUB)
        # al = al0 + SC*C1 = SC*(2 mux muy + C1)    (DVE TS)
        nc.vector.tensor_scalar(alt[:, :W2], al0[:, :W2], 1.0, C1S, MUL, ADD)
        # rd = S4' - u0 = SC*(sigxx + sigyy + C2)   (GPSIMD, after u0)
        nc.vector.tensor_tensor(rdt[:, :W2], S4, u0t[:, :W2], SUB)
        # u1 = u0 + SC*C1 = SC*(mux^2+muy^2 + C1)   (DVE TS)
        nc.vector.tensor_scalar(u1t[:, :W2], u0t[:, :W2], 1.0, C1S, MUL, ADD)
        # num = al * rn                              (DVE)
        nc.vector.tensor_tensor(numt[:, :W2], alt[:, :W2], rnt[:, :W2], MUL)
        # den = u1 * rd                              (DVE)
        nc.vector.tensor_tensor(dent[:, :W2], u1t[:, :W2], rdt[:, :W2], MUL)
        if t0 + nt == NT:
            # last output tile (t=117): only 54 valid wc' partitions.
            # num <- num*m ; den <- den*m + (1-m)  (m: 1 valid / 0 invalid)
            iv0 = NV * (NT - 1 - t0)
            nc.vector.tensor_scalar(
                numt[:TW, iv0 : iv0 + NV], numt[:TW, iv0 : iv0 + NV],
                msk_s[:TW, 0:1], 0.0, MUL, ADD,
            )
            nc.vector.tensor_scalar(
                dent[:TW, iv0 : iv0 + NV], dent[:TW, iv0 : iv0 + NV],
                msk_s[:TW, 0:1], msk_s[:TW, 1:2], MUL, ADD,
            )
        # rec = 1/den                                (ACT Reciprocal, raw)
        _act_raw(nc.scalar, rect[:, :W2], dent[:, :W2], ACTF.Reciprocal)
        # ssim = num * rec; acc[:, grp] += sum      (DVE TT + TS accum)
        nc.vector.tensor_tensor(numt[:, :W2], numt[:, :W2], rect[:, :W2], MUL)
        nc.vector.tensor_scalar(
            al0[:, :W2], numt[:, :W2], 1.0, 0.0, MUL, ADD,
            accum_out=acc[:TW, grp : grp + 1],
        )

    nc.sync.dma_start(out=acc_out[:, :], in_=acc)

    for p in (pwp, sbp, p2p, pvp, tvp, inp, consts):
        p.release()


_CACHE = {}


def _get_compiled():
    if "nc" in _CACHE:
        return _CACHE["nc"], _CACHE["aps"]
    nc = bacc.Bacc("TRN2", target_bir_lowering=False, debug=False, num_devices=NCORES)
    ins = {}
    for f in FIELDS:
        ins[f"f{f}"] = nc.dram_tensor(
            f"f{f}", [HSLAB, WPAD], F16, kind="ExternalInput"
        ).ap()
    ins["bv"] = nc.dram_tensor("bv", [128, 118], F16, kind="ExternalInput").ap()
    ins["bh"] = nc.dram_tensor("bh", [128, TW], F16, kind="ExternalInput").ap()
    ins["msk"] = nc.dram_tensor("msk", [128, 2], F32, kind="ExternalInput").ap()
    outs = {"acc": nc.dram_tensor("acc", [128, NB], F32, kind="ExternalOutput").ap()}
    with tile.TileContext(nc) as tc:
        _body(tc, ins, outs)
    nc.compile()
    _CACHE["nc"] = nc
    _CACHE["aps"] = (ins, outs)
    return nc, (ins, outs)


LAST_RES = None


def kernel(X, Y, _trace=False, _trace_kwargs=None):
    global LAST_RES
    X = np.asarray(X, dtype=np.float32).reshape(H, WC)
    Y = np.asarray(Y, dtype=np.float32).reshape(H, WC)

    bv, bh = _bands()
    nvalid = WC - TW * (NT - 1)  # 54
    msk = np.zeros((128, 2), dtype=np.float32)
    msk[:nvalid, 0] = 1.0
    msk[:, 1] = 1.0 - msk[:, 0]
    fields = {
        "fp": X + Y,
        "fm": X - Y,
        "fq": 2.0 * (X * Y),
        "fs": X * X + Y * Y,
    }
    # pad rows (5 top/bottom) and wc (15 left, to WPAD right), cast fp16
    padded = {}
    for k, a in fields.items():
        p = np.zeros((H + 10, WPAD), dtype=np.float16)
        p[5 : 5 + H, PADL : PADL + WC] = a.astype(np.float16)
        padded[k] = p

    in_maps = []
    for c in range(NCORES):
        m = {
            k: np.ascontiguousarray(p[HOUT * c : HOUT * c + HSLAB])
            for k, p in padded.items()
        }
        m["bv"] = bv
        m["bh"] = bh
        m["msk"] = msk
        in_maps.append(m)

    nc, _ = _get_compiled()
    res = run_bass_kernel_spmd(
        nc, in_maps, core_ids=list(range(NCORES)),
        trace=_trace, **(_trace_kwargs or {}),
    )
    LAST_RES = res
    total = 0.0
    for r in res.results:
        total += float(np.asarray(r["acc"])[:TW, :].astype(np.float64).sum())
    # acc holds SC*num/(SC*den) = ssim, summed over all pixels
    loss = 1.0 - total / (H * W * C)
    return np.float32(loss)
